# revision 19
# baseline (speedup 1.0000x reference)
"""Trainium2 Bass kernel for nn_CRA_46797963657479.

Math: the reference builds per-batch gram matrix A = cat_phi^T cat_phi
([B,392,392]) and feeds concat(A, A^T) through big 1x1 convs.  Since A is
symmetric and everything after cat_phi is linear, the whole tail collapses:

    W[b, l] = (u3 + cat_phi[b] @ u4) . cat_phi[b][:, l] + K
    out[b]  = xp[b] * W[b, :N] + yp[b] * W[b, N:]

with u3 = w5a @ w3, u4 = w5b @ (w4[:, :2N] + w4[:, 2N:]),
K = w5a.b3 + w5b.b4 + b5.  BN folds into the conv weights.  What remains per
batch is two 192x192 matmuls (phi_x, phi_y), a weighted free-dim reduction
(z), one more matmul for W, and an elementwise combine -> memory-bound.

Sharding: pure data parallel, batch 256 -> 32 per core on 8 cores.
"""

import os
import ml_dtypes
import numpy as np

import concourse.bass as bass
import concourse.bacc as bacc
import concourse.tile as tile
from concourse import mybir
from concourse.bass_utils import run_bass_kernel_spmd

F32 = mybir.dt.float32
F32R = mybir.dt.float32r
BF16 = mybir.dt.bfloat16

B, N, C = 256, 196, 192
NCORES = 8
NB = B // NCORES          # 32 batches per core
NPAIR = NB // 2           # 16 pairs per core
L = 2 * N                 # 392 free columns per pair tile / per stream-pack
CLO, CHI = 128, C - 128   # 128 + 64 channel split
CHIA = CHI + 1            # hi chunk augmented with a ones-row (folds +K)
CB_COLS = 1562            # const blob columns

_CACHE = {}


def _build_program():
    nc = bacc.Bacc("TRN2", target_bir_lowering=False, debug=False)

    xy = nc.dram_tensor("xy", [NB, C, 2, N], F32R, kind="ExternalInput")
    out = nc.dram_tensor("out", [NB, C, N], F32, kind="ExternalOutput")
    # all constants packed into one blob: [128 partitions, 1562 f32 cols]
    cblob = nc.dram_tensor("cblob", [CLO, CB_COLS], F32R, kind="ExternalInput")

    xyc = xy.rearrange("b c s n -> c b s n")     # [C, NB, 2, N]
    outv = out.rearrange("b c n -> c b n")       # [C, NB, N]

    with tile.TileContext(nc) as tc:
        with (
            tc.tile_pool(name="consts", bufs=1) as consts,
            tc.tile_pool(name="xin", bufs=3) as xin,
            tc.tile_pool(name="phi", bufs=3) as phip,
            tc.tile_pool(name="junk", bufs=3) as junkp,
            tc.tile_pool(name="qp", bufs=3) as qp,
            tc.tile_pool(name="work", bufs=3) as work,
            tc.tile_pool(name="outp", bufs=3) as outp,
            tc.tile_pool(name="psph", bufs=1, space="PSUM") as psph,
            tc.tile_pool(name="psw", bufs=2, space="PSUM") as psw,
        ):
            blob = consts.tile([CLO, CB_COLS], F32R)
            nc.scalar.dma_start(out=blob[:], in_=cblob[:])

            def bv(c0, ncols, rows=CLO, dt=F32):
                ap = blob[0:rows, c0:c0 + ncols]
                return ap if dt is F32R else ap.bitcast(dt)

            twxa = bv(0, CLO, dt=F32R)
            twxb = bv(128, CHIA, dt=F32R)
            twxc = bv(193, CLO, rows=CHI, dt=F32R)
            twxd = bv(321, CHIA, rows=CHI, dt=F32R)
            twya = bv(386, CLO, dt=F32R)
            twyb = bv(514, CHIA, dt=F32R)
            twyc = bv(579, CLO, rows=CHI, dt=F32R)
            twyd = bv(707, CHIA, rows=CHI, dt=F32R)
            tc1lo = bv(772, 1)
            tc1hi = bv(773, 1, rows=CHIA)
            tc2lo = bv(774, 1)
            tc2hi = bv(775, 1, rows=CHIA)
            tu4lo = bv(776, L)
            tu4hi = bv(1168, L, rows=CHIA)
            tu3lo = bv(1560, 1)
            tu3hi = bv(1561, 1, rows=CHIA)

            def f(ap):
                return ap.bitcast(F32)

            relu = mybir.ActivationFunctionType.Relu
            mult = mybir.AluOpType.mult
            add = mybir.AluOpType.add
            byp = mybir.AluOpType.bypass

            GP = 2  # pairs per I/O group
            for g in range(NPAIR // GP):
              gb = 2 * GP * g
              # group loads: [channels, chunk, batch, stream, N]
              xg = xin.tile([CLO, 2, 2 * GP, 2, N], F32R)
              nc.gpsimd.memset(xg[CHI:CLO, 1, :, :, :].bitcast(F32), 0.0)
              nc.sync.dma_start(out=xg[:, 0, :, :, :], in_=xyc[0:CLO, gb:gb + 2 * GP, :, :])
              nc.sync.dma_start(out=xg[0:CHI, 1, :, :, :], in_=xyc[CLO:C, gb:gb + 2 * GP, :, :])
              og = outp.tile([CLO, 2, 2 * GP, N], F32)
              for u in range(GP):
                b0 = 2 * u
                xlo2 = xg[:, 0, b0:b0 + 2, 0, :]
                xhi2 = xg[0:CHI, 1, b0:b0 + 2, 0, :]
                ylo2 = xg[:, 0, b0:b0 + 2, 1, :]
                yhi2 = xg[0:CHI, 1, b0:b0 + 2, 1, :]

                # ---- phi matmuls (pair-packed, 392 moving cols, f32r) ----
                ps_xlo = psph.tile([CLO, L], F32)
                ps_xhi = psph.tile([CHIA, L], F32)
                ps_ylo = psph.tile([CLO, L], F32)
                ps_yhi = psph.tile([CHIA, L], F32)
                nc.tensor.matmul(ps_xlo[:], twxa, xlo2, start=True, stop=False)
                nc.tensor.matmul(ps_xlo[:], twxc, xhi2, start=False, stop=True)
                nc.tensor.matmul(ps_xhi[:], twxb, xlo2, start=True, stop=False)
                nc.tensor.matmul(ps_xhi[:], twxd, xhi2, start=False, stop=True)
                nc.tensor.matmul(ps_ylo[:], twya, ylo2, start=True, stop=False)
                nc.tensor.matmul(ps_ylo[:], twyc, yhi2, start=False, stop=True)
                nc.tensor.matmul(ps_yhi[:], twyb, ylo2, start=True, stop=False)
                nc.tensor.matmul(ps_yhi[:], twyd, yhi2, start=False, stop=True)

                # ---- relu+bias -> bf16 phi, repacked per batch [phi_x | phi_y] ----
                phiA_lo = phip.tile([CLO, L], BF16)
                phiB_lo = phip.tile([CLO, L], BF16)
                phiA_hi = phip.tile([CHIA, L], BF16)
                phiB_hi = phip.tile([CHIA, L], BF16)
                nc.scalar.activation(phiA_lo[:, 0:N], ps_xlo[:, 0:N], relu, bias=tc1lo)
                nc.scalar.activation(phiA_lo[:, N:L], ps_ylo[:, 0:N], relu, bias=tc2lo)
                nc.scalar.activation(phiB_lo[:, 0:N], ps_xlo[:, N:L], relu, bias=tc1lo)
                nc.scalar.activation(phiB_lo[:, N:L], ps_ylo[:, N:L], relu, bias=tc2lo)
                nc.scalar.activation(phiA_hi[:, 0:N], ps_xhi[:, 0:N], relu, bias=tc1hi)
                nc.scalar.activation(phiA_hi[:, N:L], ps_yhi[:, 0:N], relu, bias=tc2hi)
                nc.scalar.activation(phiB_hi[:, 0:N], ps_xhi[:, N:L], relu, bias=tc1hi)
                nc.scalar.activation(phiB_hi[:, N:L], ps_yhi[:, N:L], relu, bias=tc2hi)

                # ---- z = phi @ u4 (fused mult+reduce on DVE) ----
                jA_lo = junkp.tile([CLO, L], F32, tag="j_lo")
                jB_lo = junkp.tile([CLO, L], F32, tag="j_lo")
                jA_hi = junkp.tile([CHIA, L], F32, tag="j_hi")
                jB_hi = junkp.tile([CHIA, L], F32, tag="j_hi")
                zA_lo = qp.tile([CLO, 1], F32)
                zB_lo = qp.tile([CLO, 1], F32)
                zA_hi = qp.tile([CHIA, 1], F32)
                zB_hi = qp.tile([CHIA, 1], F32)
                nc.vector.scalar_tensor_tensor(
                    out=jA_lo[:], in0=phiA_lo[:], scalar=1.0, in1=tu4lo,
                    op0=byp, op1=mult, accum_out=zA_lo[:])
                nc.vector.scalar_tensor_tensor(
                    out=jB_lo[:], in0=phiB_lo[:], scalar=1.0, in1=tu4lo,
                    op0=byp, op1=mult, accum_out=zB_lo[:])
                nc.vector.scalar_tensor_tensor(
                    out=jA_hi[:], in0=phiA_hi[:], scalar=1.0, in1=tu4hi,
                    op0=byp, op1=mult, accum_out=zA_hi[:])
                nc.vector.scalar_tensor_tensor(
                    out=jB_hi[:], in0=phiB_hi[:], scalar=1.0, in1=tu4hi,
                    op0=byp, op1=mult, accum_out=zB_hi[:])

                # ---- q = bf16(z + u3), tiny per-partition converts (gpsimd) ----
                qA_lo = qp.tile([CLO, 1], BF16, tag="qb_lo")
                qB_lo = qp.tile([CLO, 1], BF16, tag="qb_lo")
                qA_hi = qp.tile([CHIA, 1], BF16, tag="qb_hi")
                qB_hi = qp.tile([CHIA, 1], BF16, tag="qb_hi")
                nc.gpsimd.tensor_scalar(qA_lo[:], zA_lo[:], tu3lo, None, add)
                nc.gpsimd.tensor_scalar(qB_lo[:], zB_lo[:], tu3lo, None, add)
                nc.gpsimd.tensor_scalar(qA_hi[:], zA_hi[:], tu3hi, None, add)
                nc.gpsimd.tensor_scalar(qB_hi[:], zB_hi[:], tu3hi, None, add)

                # ---- W rows (bf16 matmul, stride-0 broadcast stationary) ----
                ps_w = psw.tile([CLO, 2, 512], F32)
                nc.tensor.matmul(ps_w[:, 0, 0:L], qA_lo[:].broadcast_to([CLO, CLO]),
                                 phiA_lo[:], start=True, stop=False)
                nc.tensor.matmul(ps_w[:, 0, 0:L], qA_hi[:].broadcast_to([CHIA, CLO]),
                                 phiA_hi[:], start=False, stop=True)
                nc.tensor.matmul(ps_w[:, 1, 0:L], qB_lo[:].broadcast_to([CLO, CLO]),
                                 phiB_lo[:], start=True, stop=False)
                nc.tensor.matmul(ps_w[:, 1, 0:L], qB_hi[:].broadcast_to([CHIA, CLO]),
                                 phiB_hi[:], start=False, stop=True)

                # ---- out = xp*Wx + yp*Wy; lo+hi chunks packed in one op ----
                # W is partition-broadcast, so a stride-0 chunk dim reuses it
                wx = ps_w[:, :, 0:N]
                wxb2 = bass.AP(tensor=wx.tensor, offset=wx.offset,
                               ap=[wx.ap[0], [0, 2]] + list(wx.ap[1:]))
                wy = ps_w[:, :, N:L]
                wyb2 = bass.AP(tensor=wy.tensor, offset=wy.offset,
                               ap=[wy.ap[0], [0, 2]] + list(wy.ap[1:]))
                t1 = work.tile([CLO, 2, 2, N], F32, tag="t1")
                t2 = work.tile([CLO, 2, 2, N], F32, tag="t2")
                nc.vector.tensor_mul(t1[:], f(xg[:, :, b0:b0 + 2, 0, :]), wxb2)
                nc.vector.tensor_mul(t2[:], f(xg[:, :, b0:b0 + 2, 1, :]), wyb2)
                nc.vector.tensor_add(og[:, :, b0:b0 + 2, :], t1[:], t2[:])
              nc.scalar.dma_start(out=outv[0:CLO, gb:gb + 2 * GP, :], in_=og[:, 0, :, :])
              nc.scalar.dma_start(out=outv[CLO:C, gb:gb + 2 * GP, :], in_=og[0:CHI, 1, :, :])

    nc.compile()
    return nc


def _host_prepack(d):
    """Fold BN, collapse the linear tail, build per-core constant arrays."""
    f = np.float32
    inv1 = d["g1"] / np.sqrt(d["v1"] + 1e-5)
    W1 = (d["w1"] * inv1[:, None]).astype(f)
    c1 = ((d["b1"] - d["m1"]) * inv1 + d["be1"]).astype(f)
    inv2 = d["g2"] / np.sqrt(d["v2"] + 1e-5)
    W2 = (d["w2"] * inv2[:, None]).astype(f)
    c2 = ((d["b2"] - d["m2"]) * inv2 + d["be2"]).astype(f)

    w4eff = d["w4"][:, :L] + d["w4"][:, L:]
    w5a, w5b = d["w5"][0, :C], d["w5"][0, C:]
    u3 = (w5a @ d["w3"]).astype(f)
    u4 = (w5b @ w4eff).astype(f)
    K = float(w5a @ d["b3"] + w5b @ d["b4"] + d["b5"][0])

    W1T, W2T = np.ascontiguousarray(W1.T), np.ascontiguousarray(W2.T)

    def hi_pad_m(a):  # [k, 64] -> [k, 65] with zero last col
        z = np.zeros((a.shape[0], CHIA), f)
        z[:, :CHI] = a
        return z

    blob = np.zeros((CLO, CB_COLS), f)

    def put(c0, arr, rows=None):
        a = np.asarray(arr, f)
        r = a.shape[0]
        blob[:r, c0:c0 + a.shape[1]] = a

    put(0, W1T[:CLO, :CLO])
    put(128, hi_pad_m(W1T[:CLO, CLO:C]))
    put(193, W1T[CLO:C, :CLO])
    put(321, hi_pad_m(W1T[CLO:C, CLO:C]))
    put(386, W2T[:CLO, :CLO])
    put(514, hi_pad_m(W2T[:CLO, CLO:C]))
    put(579, W2T[CLO:C, :CLO])
    put(707, hi_pad_m(W2T[CLO:C, CLO:C]))
    put(772, c1[:CLO, None])
    put(773, np.concatenate([c1[CLO:C], [f(1.0)]])[:, None])
    put(774, c2[:CLO, None])
    put(775, np.concatenate([c2[CLO:C], [f(1.0)]])[:, None])
    put(776, np.broadcast_to(u4, (CLO, L)))
    put(1168, np.concatenate([np.broadcast_to(u4, (CHI, L)),
                              np.zeros((1, L), f)], axis=0))
    put(1560, u3[:CLO, None])
    put(1561, np.concatenate([u3[CLO:C], [f(K)]])[:, None])
    return {"cblob": blob}


def run(inputs, trace=False):
    d = {k: np.asarray(v) for k, v in inputs.items()}
    consts = _host_prepack(d)

    xyp = np.empty((B, C, 2, N), np.float32)
    xyp[:, :, 0] = d["x"].transpose(0, 2, 1)
    xyp[:, :, 1] = d["y"].transpose(0, 2, 1)

    if "nc" not in _CACHE:
        _CACHE["nc"] = _build_program()
    nc = _CACHE["nc"]

    in_maps = []
    for cid in range(NCORES):
        m = dict(consts)
        m["xy"] = np.ascontiguousarray(xyp[cid * NB:(cid + 1) * NB])
        in_maps.append(m)

    res = run_bass_kernel_spmd(nc, in_maps, list(range(NCORES)), trace=trace)
    out = np.concatenate([res.results[i]["out"] for i in range(NCORES)], axis=0)
    return out, res


def kernel(**inputs):
    out, _ = run(inputs, trace=False)
    return out
